# revision 4
# baseline (speedup 1.0000x reference)
"""Bahdanau attention Trainium2 kernel.

Problem shapes (fixed): B=64, T=1024, KS=QS=H=1024, fp32.
  proj_keys = keys @ W_key                  [B,T,H]
  q         = query @ W_query               [B,1,H]
  scores    = tanh(q + proj_keys) . w_score [B,T]
  alphas    = softmax(mask(scores))         [B,1,T]
  context   = alphas @ values               [B,1,KS]

Sharding: data-parallel over batch across 8 NeuronCores (8 batches/core),
weights replicated.

Per-core dataflow (per batch):
  - load keys naturally [T,KS], PE-transpose 128x128 tiles -> keysT [KS,T]
  - projT[h,t] = sum_k W_key[k,h]*keysT[k,t]: stationary = W_key tile
    (natural layout), moving = keysT, fp32r matmuls (full PE rate at N>=256)
  - ScalarE: S = tanh(projT + q[h]) fused PSUM->SBUF with per-partition bias
  - scores[1,T] = w_score^T @ S (w_score stationary [128,1])
  - softmax on a single partition row: reduce_max(negated) -> Exp activation
    with bias=-max and accum_out=sum -> reciprocal -> scale
  - alphas row -> columns via 8 tiny PE transposes
  - context[1,KS] = sum_t alphas[t] * values[t,:]: stationary = alphas column
    [128,1], moving = values (natural layout), fp32r
"""

import numpy as np

import concourse.bass as bass
import concourse.mybir as mybir
import concourse.tile as tile
from concourse.masks import make_identity

f32 = mybir.dt.float32
f32r = mybir.dt.float32r

P = 128        # partitions
TB = 8         # batches per core
T = 1024       # sequence length
H = 1024       # hidden (= KS = QS)
NC_ = 8        # chunks of 128 along T/H/KS
NH = 512       # matmul moving free-dim (fp32 max, one PSUM bank)

AX = mybir.AxisListType
ALU = mybir.AluOpType
ACT = mybir.ActivationFunctionType


def _split_drain_waits(nc, max_waits: int = 1):
    """walrus CTRL encoding supports a limited number of sem waits per
    instruction; Tile's final drain can carry many.  Hoist extras onto
    preceding single-wait drains."""
    for func in nc.m.functions:
        for blk in func.blocks:
            new_insts = []
            for inst in blk.instructions:
                si = inst.sync_info
                if si is not None and si.on_wait and len(si.on_wait) > max_waits:
                    waits = list(si.on_wait)
                    extra, keep = waits[:-max_waits], waits[-max_waits:]
                    for j, w in enumerate(extra):
                        new_insts.append(
                            mybir.InstDrain(
                                name=f"{inst.name}-presplit{j}",
                                engine=inst.engine,
                                sync_info=mybir.SyncInfo(on_wait=[w], on_update=[]),
                            )
                        )
                    si.on_wait = keep
                new_insts.append(inst)
            blk.instructions = new_insts


def build_bahdanau_nc(split_drains=True):
    """Build the per-core Bass program (identical on all 8 cores)."""
    import contextlib

    nc = bass.Bass(trn_type="TRN2", target_bir_lowering=False, debug=False)

    keys_d = nc.dram_tensor("keys", [TB, T, H], f32, kind="ExternalInput").ap()
    values_d = nc.dram_tensor("values", [TB, T, H], f32, kind="ExternalInput").ap()
    wkey_d = nc.dram_tensor("wkey", [H, H], f32, kind="ExternalInput").ap()
    wquery_d = nc.dram_tensor("wquery", [H, H], f32, kind="ExternalInput").ap()
    # queryt: host-prearranged query^T as [p, kchunk, b]
    qtin_d = nc.dram_tensor("qtin", [P, NC_, TB], f32, kind="ExternalInput").ap()
    # w_score host-prearranged as [p, kchunk]
    wsc_d = nc.dram_tensor("wsc", [P, NC_], f32, kind="ExternalInput").ap()
    # additive mask bias (0 where visible, -1e30 where masked)
    maskb_d = nc.dram_tensor("maskb", [TB, T], f32, kind="ExternalInput").ap()

    ctx_d = nc.dram_tensor("ctx", [TB, H], f32, kind="ExternalOutput").ap()
    alph_d = nc.dram_tensor("alph", [TB, T], f32, kind="ExternalOutput").ap()

    with tile.TileContext(nc) as tc, contextlib.ExitStack() as ctx:
        # ---- persistent pools
        const_pool = ctx.enter_context(tc.tile_pool(name="const", bufs=1))
        knat_pool = ctx.enter_context(tc.tile_pool(name="knat", bufs=3))
        ktr_pool = ctx.enter_context(tc.tile_pool(name="ktr", bufs=2))
        s_pool = ctx.enter_context(tc.tile_pool(name="spool", bufs=3))
        v_pool = ctx.enter_context(tc.tile_pool(name="vpool", bufs=3))
        row_pool = ctx.enter_context(tc.tile_pool(name="rows", bufs=2))
        small_pool = ctx.enter_context(tc.tile_pool(name="small", bufs=2))

        tr_psum = ctx.enter_context(tc.tile_pool(name="trps", bufs=2, space="PSUM"))
        ps_pool = ctx.enter_context(tc.tile_pool(name="psS", bufs=2, space="PSUM"))
        sm_psum = ctx.enter_context(tc.tile_pool(name="smps", bufs=2, space="PSUM"))

        # ---- preamble: first batch's keys, weights, identity, q projection
        ident = const_pool.tile([P, P], f32, tag="ident", name="ident")
        make_identity(nc, ident[:, :])

        knat0 = []
        for t in range(NC_):
            kn = knat_pool.tile([P, H], f32, tag="knat", name=f"knat0_{t}")
            nc.sync.dma_start(kn[:, :], keys_d[0, t * P : (t + 1) * P, :])
            knat0.append(kn)

        # W_key: stage fp32 then round to fp32r via DVE copy (fp32r matmul
        # operands must be produced pre-rounded)
        wk = const_pool.tile([P, NC_, H], f32r, tag="wk", name="wk")
        for k in range(NC_):
            wst = knat_pool.tile([P, H], f32, tag="wstage", bufs=2, name=f"wst{k}")
            nc.sync.dma_start(wst[:, :], wkey_d[k * P : (k + 1) * P, :])
            nc.vector.tensor_copy(wk[:, k, :], wst[:, :])

        wq = const_pool.tile([P, NC_, H], f32, tag="wq", name="wq")
        for k in range(NC_):
            nc.sync.dma_start(wq[:, k, :], wquery_d[k * P : (k + 1) * P, :])
        qtin = const_pool.tile([P, NC_, TB], f32, tag="qtin", name="qtin")
        nc.sync.dma_start(qtin[:, :, :], qtin_d[:, :, :])
        wsc_raw = const_pool.tile([P, NC_], f32, tag="wsc_raw", name="wsc_raw")
        nc.sync.dma_start(wsc_raw[:, :], wsc_d[:, :])
        wsc = const_pool.tile([P, NC_], f32r, tag="wsc", name="wsc")
        nc.vector.tensor_copy(wsc[:, :], wsc_raw[:, :])

        # q projection: qT[h, b] per h-chunk, exact fp32
        qT = const_pool.tile([P, NC_, TB], f32, tag="qT", name="qT")
        for m in range(NC_):
            psq = tr_psum.tile([P, TB], f32, tag="tr", name=f"psq{m}")
            for k in range(NC_):
                nc.tensor.matmul(
                    psq[:, :],
                    lhsT=wq[:, k, m * P : (m + 1) * P],
                    rhs=qtin[:, k, :],
                    start=(k == 0),
                    stop=(k == NC_ - 1),
                )
            nc.scalar.copy(qT[:, m, :], psq[:, :])

        # ---- batch loop
        for b in range(TB):
            if b == 0:
                knat = knat0
            else:
                knat = []
                for t in range(NC_):
                    kn = knat_pool.tile([P, H], f32, tag="knat", name=f"knat{b}_{t}")
                    nc.sync.dma_start(kn[:, :], keys_d[b, t * P : (t + 1) * P, :])
                    knat.append(kn)

            mb = small_pool.tile([1, T], f32, tag="mb", name=f"mb{b}")
            nc.sync.dma_start(mb[:, :], maskb_d[b : b + 1, :])

            # transpose keys -> keysT [k-chunk partition, t free]
            ktr = ktr_pool.tile([P, NC_, T], f32r, tag="ktr", name=f"ktr{b}")
            for t in range(NC_):
                for k in range(NC_):
                    ptr = tr_psum.tile([P, P], f32, tag="tr", name=f"ptr{b}_{t}_{k}")
                    nc.tensor.transpose(
                        ptr[:, :], knat[t][:, k * P : (k + 1) * P], ident[:, :]
                    )
                    nc.vector.tensor_copy(ktr[:, k, t * P : (t + 1) * P], ptr[:, :])

            # main matmul + tanh + scores
            psc = [
                sm_psum.tile([1, NH], f32, tag="sm", name=f"psc{b}_{n}")
                for n in range(2)
            ]
            for m in range(NC_):
                ps = ps_pool.tile([P, T], f32, tag="ps", name=f"ps{b}_{m}")
                for k in range(NC_):
                    for n in range(2):
                        nc.tensor.matmul(
                            ps[:, n * NH : (n + 1) * NH],
                            lhsT=wk[:, k, m * P : (m + 1) * P],
                            rhs=ktr[:, k, n * NH : (n + 1) * NH],
                            start=(k == 0),
                            stop=(k == NC_ - 1),
                        )
                s = s_pool.tile([P, T], f32r, tag="s", name=f"s{b}_{m}")
                nc.scalar.activation(
                    s[:, :], ps[:, :], ACT.Tanh, bias=qT[:, m, b : b + 1]
                )
                for n in range(2):
                    nc.tensor.matmul(
                        psc[n][:, :],
                        lhsT=wsc[:, m : m + 1],
                        rhs=s[:, n * NH : (n + 1) * NH],
                        start=(m == 0),
                        stop=(m == NC_ - 1),
                    )

            # prefetch values
            vts = []
            for t in range(NC_):
                vt = v_pool.tile([P, H], f32r, tag="v", name=f"v{b}_{t}")
                nc.gpsimd.dma_start(vt[:, :], values_d[b, t * P : (t + 1) * P, :])
                vts.append(vt)

            # mask add (PSUM->SBUF), softmax on one partition
            sc = row_pool.tile([1, T], f32, tag="sc", name=f"sc{b}")
            for n in range(2):
                nc.vector.tensor_add(
                    sc[:, n * NH : (n + 1) * NH],
                    psc[n][:, :],
                    mb[:, n * NH : (n + 1) * NH],
                )
            nmax = small_pool.tile([1, 1], f32, tag="nmax", name=f"nmax{b}")
            nc.vector.tensor_reduce(
                nmax[:, :], sc[:, :], axis=AX.X, op=ALU.max, negate=True
            )
            arow = row_pool.tile([1, T], f32, tag="arow", name=f"arow{b}")
            ssum = small_pool.tile([1, 1], f32, tag="ssum", name=f"ssum{b}")
            nc.scalar.activation(
                arow[:, :],
                sc[:, :],
                ACT.Exp,
                bias=nmax[:, :],
                accum_out=ssum[:, :],
            )
            rinv = small_pool.tile([1, 1], f32, tag="rinv", name=f"rinv{b}")
            nc.vector.reciprocal(rinv[:, :], ssum[:, :])
            nc.vector.tensor_scalar_mul(arow[:, :], arow[:, :], rinv[:, :])
            nc.sync.dma_start(alph_d[b : b + 1, :], arow[:, :])

            # alphas row -> columns
            paT = tr_psum.tile([P, TB], f32, tag="tr", name=f"paT{b}")
            for k in range(NC_):
                nc.tensor.transpose(
                    paT[:, k : k + 1],
                    arow[0:1, k * P : (k + 1) * P],
                    ident[0:1, 0:1],
                )
            aT = small_pool.tile([P, NC_], f32r, tag="aT", name=f"aT{b}")
            nc.vector.tensor_copy(aT[:, :], paT[:, :])

            # context
            pcx = [
                sm_psum.tile([1, NH], f32, tag="sm", name=f"pcx{b}_{n}")
                for n in range(2)
            ]
            for k in range(NC_):
                for n in range(2):
                    nc.tensor.matmul(
                        pcx[n][:, :],
                        lhsT=aT[:, k : k + 1],
                        rhs=vts[k][:, n * NH : (n + 1) * NH],
                        start=(k == 0),
                        stop=(k == NC_ - 1),
                    )
            cxr = row_pool.tile([1, H], f32, tag="cxr", name=f"cxr{b}")
            for n in range(2):
                nc.scalar.copy(cxr[:, n * NH : (n + 1) * NH], pcx[n][:, :])
            nc.sync.dma_start(ctx_d[b : b + 1, :], cxr[:, :])

    if split_drains:
        _split_drain_waits(nc)
    return nc


_NC_CACHE = None


def _get_nc():
    global _NC_CACHE
    if _NC_CACHE is None:
        _NC_CACHE = build_bahdanau_nc()
    return _NC_CACHE


def make_in_maps(query, mask, values, keys, W_key, W_query, w_score):
    """Shard full inputs into per-core input maps (host-side layout only)."""
    query = np.ascontiguousarray(np.asarray(query, dtype=np.float32))
    mask = np.asarray(mask)
    values = np.ascontiguousarray(np.asarray(values, dtype=np.float32))
    keys = np.ascontiguousarray(np.asarray(keys, dtype=np.float32))
    W_key = np.ascontiguousarray(np.asarray(W_key, dtype=np.float32))
    W_query = np.ascontiguousarray(np.asarray(W_query, dtype=np.float32))
    w_score = np.ascontiguousarray(np.asarray(w_score, dtype=np.float32))

    B = query.shape[0]
    n_cores = B // TB
    maskb = np.where(mask, np.float32(0.0), np.float32(-1e30)).astype(np.float32)
    wsc_in = np.ascontiguousarray(w_score.reshape(NC_, P).T)

    in_maps = []
    for c in range(n_cores):
        sl = slice(c * TB, (c + 1) * TB)
        qt = query[sl, 0, :].T  # [QS, TB]
        qtin = np.ascontiguousarray(qt.reshape(NC_, P, TB).transpose(1, 0, 2))
        in_maps.append(
            {
                "keys": keys[sl],
                "values": values[sl],
                "wkey": W_key,
                "wquery": W_query,
                "qtin": qtin,
                "wsc": wsc_in,
                "maskb": np.ascontiguousarray(maskb[sl]),
            }
        )
    return in_maps


def kernel(query, mask, values, keys, W_key, W_query, w_score):
    from concourse.bass_utils import run_bass_kernel_spmd

    B = np.asarray(query).shape[0]
    n_cores = B // TB
    in_maps = make_in_maps(query, mask, values, keys, W_key, W_query, w_score)
    nc = _get_nc()
    res = run_bass_kernel_spmd(nc, in_maps, core_ids=list(range(n_cores)))
    context = np.concatenate([r["ctx"] for r in res.results], axis=0)
    alphas = np.concatenate([r["alph"] for r in res.results], axis=0)
    return context.reshape(B, 1, H), alphas.reshape(B, 1, T)


# revision 21
# speedup vs baseline: 150.5395x; 150.5395x over previous
"""Bahdanau attention Trainium2 kernel.

Problem shapes (fixed): B=64, T=1024, KS=QS=H=1024, fp32.
  proj_keys = keys @ W_key                  [B,T,H]
  q         = query @ W_query               [B,1,H]
  scores    = tanh(q + proj_keys) . w_score [B,T]
  alphas    = softmax(mask(scores))         [B,1,T]
  context   = alphas @ values               [B,1,KS]

Sharding: data-parallel over batch across 8 NeuronCores (8 batches/core),
weights replicated.

Per-core dataflow (per batch):
  - load keys naturally [T,KS], PE-transpose 128x128 tiles -> keysT [KS,T]
    (4 transposes packed per PSUM tile, single strided copy out, copies
    alternate DVE/ACT; next batch's transposes interleave with this batch's
    main matmul m-tiles to keep PE dense)
  - projT[h,t] = sum_k W_key[k,h]*keysT[k,t]: stationary = W_key tile
    (natural layout), moving = keysT, fp32r matmuls (full PE rate at N>=256)
  - ScalarE: S = tanh(projT + q[h]) fused PSUM->SBUF with per-partition bias
  - scores[1,T] = w_score^T @ S (w_score stationary [128,1])
  - softmax on a single partition row: reduce_max(negated) -> Exp activation
    with bias=-max and accum_out=sum -> reciprocal -> scale
  - alphas row -> columns via 8 tiny PE transposes
  - context[1,KS] = sum_t alphas[t] * values[t,:]: stationary = alphas column
    [128,1], moving = values (natural layout, rounded to fp32r on load)
"""

import numpy as np

import concourse.bass as bass
import concourse.mybir as mybir
import concourse.tile as tile
from concourse.masks import make_identity

f32 = mybir.dt.float32
f32r = mybir.dt.float32r

P = 128        # partitions
TB = 8         # batches per core
T = 1024       # sequence length
H = 1024       # hidden (= KS = QS)
NC_ = 8        # chunks of 128 along T/H/KS
NH = 512       # matmul moving free-dim (fp32 max, one PSUM bank)

AX = mybir.AxisListType
ALU = mybir.AluOpType
ACT = mybir.ActivationFunctionType


def _split_drain_waits(nc, max_waits: int = 1):
    """walrus CTRL encoding supports a limited number of sem waits per
    instruction; Tile's final drain can carry many.  Hoist extras onto
    preceding single-wait drains."""
    for func in nc.m.functions:
        for blk in func.blocks:
            new_insts = []
            for inst in blk.instructions:
                si = inst.sync_info
                if si is not None and si.on_wait and len(si.on_wait) > max_waits:
                    waits = list(si.on_wait)
                    extra, keep = waits[:-max_waits], waits[-max_waits:]
                    for j, w in enumerate(extra):
                        new_insts.append(
                            mybir.InstDrain(
                                name=f"{inst.name}-presplit{j}",
                                engine=inst.engine,
                                sync_info=mybir.SyncInfo(on_wait=[w], on_update=[]),
                            )
                        )
                    si.on_wait = keep
                new_insts.append(inst)
            blk.instructions = new_insts


def build_bahdanau_nc(split_drains=True, reps=1, big_io=True, mm_bf16=False):
    """Build the per-core Bass program (identical on all 8 cores)."""
    import contextlib

    nc = bass.Bass(trn_type="TRN2", target_bir_lowering=False, debug=False)

    big = "ExternalInput" if big_io else "Internal"
    keys_d = nc.dram_tensor("keys", [TB, T, H], f32, kind=big).ap()
    values_d = nc.dram_tensor("values", [TB, T, H], f32, kind=big).ap()
    wkey_d = nc.dram_tensor("wkey", [H, H], f32, kind=big).ap()
    wquery_d = nc.dram_tensor("wquery", [H, H], f32, kind=big).ap()
    # queryt: host-prearranged query^T as [p, kchunk, b]
    qtin_d = nc.dram_tensor("qtin", [P, NC_, TB], f32, kind="ExternalInput").ap()
    # w_score host-prearranged as [p, kchunk]
    wsc_d = nc.dram_tensor("wsc", [P, NC_], f32, kind="ExternalInput").ap()
    # additive mask bias (0 where visible, -1e30 where masked)
    maskb_d = nc.dram_tensor("maskb", [TB, T], f32, kind="ExternalInput").ap()

    ctx_d = nc.dram_tensor("ctx", [TB, H], f32, kind="ExternalOutput").ap()
    alph_d = nc.dram_tensor("alph", [TB, T], f32, kind="ExternalOutput").ap()

    with tile.TileContext(nc) as tc, contextlib.ExitStack() as ctx:
        # ---- pools
        const_pool = ctx.enter_context(tc.tile_pool(name="const", bufs=1))
        knat_pool = ctx.enter_context(tc.tile_pool(name="knat", bufs=2))
        ktr_pool = ctx.enter_context(tc.tile_pool(name="ktr", bufs=2))
        s_pool = ctx.enter_context(tc.tile_pool(name="spool", bufs=2))
        v_pool = ctx.enter_context(tc.tile_pool(name="vpool", bufs=3))
        row_pool = ctx.enter_context(tc.tile_pool(name="rows", bufs=3))
        small_pool = ctx.enter_context(tc.tile_pool(name="small", bufs=2))

        tr_psum = ctx.enter_context(tc.tile_pool(name="trps", bufs=2, space="PSUM"))
        ps_pool = ctx.enter_context(tc.tile_pool(name="psS", bufs=2, space="PSUM"))
        sm_psum = ctx.enter_context(tc.tile_pool(name="smps", bufs=2, space="PSUM"))

        # ---- preamble
        ident = const_pool.tile([P, P], f32, tag="ident", name="ident")
        make_identity(nc, ident[:, :])

        # prefetch ACT tables for Tanh/Exp during startup DMAs
        warm = const_pool.tile([1, 1], f32, tag="warm", name="warm")
        nc.scalar.activation(warm[:, :], ident[0:1, 0:1], ACT.Tanh)
        nc.scalar.activation(warm[:, :], ident[0:1, 0:1], ACT.Exp)

        # W_key first: stage fp32 then round via DVE copy (fp32r matmul
        # operands must be produced pre-rounded; bf16 variant halves
        # weight-load time via FWL at ~bf16 weight precision)
        mm_dt = mybir.dt.bfloat16 if mm_bf16 else f32r
        wk = const_pool.tile([P, NC_, H], mm_dt, tag="wk", name="wk")
        for k in range(NC_):
            wst = v_pool.tile([P, H], f32, tag="vraw", bufs=2, name=f"wst{k}")
            nc.sync.dma_start(wst[:, :], wkey_d[k * P : (k + 1) * P, :])
            nc.vector.tensor_copy(wk[:, k, :], wst[:, :])

        knat0 = []
        for t in range(NC_):
            kn = knat_pool.tile([P, H], f32, tag="knat", name=f"knat0_{t}")
            nc.sync.dma_start(kn[:, :], keys_d[0, t * P : (t + 1) * P, :])
            knat0.append(kn)

        qtin = const_pool.tile([P, NC_, TB], f32, tag="qtin", name="qtin")
        nc.sync.dma_start(qtin[:, :, :], qtin_d[:, :, :])
        wsc_raw = const_pool.tile([P, NC_], f32, tag="wsc_raw", name="wsc_raw")
        nc.sync.dma_start(wsc_raw[:, :], wsc_d[:, :])
        wsc = const_pool.tile([P, NC_], f32r, tag="wsc", name="wsc")
        nc.vector.tensor_copy(wsc[:, :], wsc_raw[:, :])
        wq = const_pool.tile([P, NC_, H], f32, tag="wq", name="wq")
        for k in range(NC_):
            nc.sync.dma_start(wq[:, k, :], wquery_d[k * P : (k + 1) * P, :])
        qT = const_pool.tile([P, NC_, TB], f32, tag="qT", name="qT")

        # ---- helpers
        def emit_transposes(tag, knat_t, t, ktr_dst):
            """Transpose one [128, KS] keys chunk into ktr_dst k-chunks.
            4 transposes share one PSUM tile; single strided copy out
            (also rounds to fp32r); copies alternate DVE/ACT."""
            for g in range(2):
                ptr = tr_psum.tile(
                    [P, 4 * P], f32, tag="tr", name=f"ptr{tag}_{t}_{g}"
                )
                for j in range(4):
                    k = 4 * g + j
                    nc.tensor.transpose(
                        ptr[:, j * P : (j + 1) * P],
                        knat_t[:, k * P : (k + 1) * P],
                        ident[:, :],
                    )
                src = ptr[:, :].rearrange("p (k c) -> p k c", k=4)
                dst = ktr_dst[:, 4 * g : 4 * g + 4, t * P : (t + 1) * P]
                if g == 0:
                    nc.vector.tensor_copy(dst, src)
                else:
                    nc.scalar.copy(dst, src)

        def emit_keys_chunk(b, t):
            kn = knat_pool.tile([P, H], f32, tag="knat", name=f"knat{b}_{t}")
            nc.sync.dma_start(kn[:, :], keys_d[b, t * P : (t + 1) * P, :])
            return kn

        # ---- steady-state batch pipeline (reps>1 repeats for timing only)
        for rep in range(reps):
            ktr_cur = ktr_pool.tile([P, NC_, T], mm_dt, tag="ktr", name="ktr_b0")
            mb_cur = small_pool.tile([1, T], f32, tag="mb", name="mb_b0")
            nc.sync.dma_start(mb_cur[:, :], maskb_d[0:1, :])
            if rep == 0:
                kn_first = knat0
            else:
                kn_first = [emit_keys_chunk(0, t) for t in range(NC_)]
            for t in range(NC_):
                emit_transposes(f"r{rep}b0", kn_first[t][:, :], t, ktr_cur)

            if rep == 0:
                # q projection (exact fp32) emitted after b0 transposes so
                # the PE stream is not head-blocked on the W_query DMAs;
                # all 64 [h,b] columns accumulate in one PSUM tile
                psq = tr_psum.tile([P, NC_ * TB], f32, tag="tr", name="psq")
                for m in range(NC_):
                    for k in range(NC_):
                        nc.tensor.matmul(
                            psq[:, m * TB : (m + 1) * TB],
                            lhsT=wq[:, k, m * P : (m + 1) * P],
                            rhs=qtin[:, k, :],
                            start=(k == 0),
                            stop=(k == NC_ - 1),
                        )
                nc.scalar.copy(qT[:, :, :], psq[:, :].rearrange("p (m b) -> p m b", m=NC_))

            for b in range(TB):
                last = b == TB - 1
                # values prefetch + round to fp32r (consumed by ctx matmul)
                vts = []
                for t in range(NC_):
                    vraw = v_pool.tile(
                        [P, H], f32, tag="vraw", bufs=2, name=f"vraw{b}_{t}"
                    )
                    nc.sync.dma_start(vraw[:, :], values_d[b, t * P : (t + 1) * P, :])
                    vt = v_pool.tile([P, H], f32r, tag="v", bufs=8, name=f"v{b}_{t}")
                    nc.vector.tensor_copy(vt[:, :], vraw[:, :])
                    vts.append(vt)

                if not last:
                    ktr_next = ktr_pool.tile(
                        [P, NC_, T], mm_dt, tag="ktr", name=f"ktr_b{b + 1}"
                    )
                    mb_next = small_pool.tile(
                        [1, T], f32, tag="mb", name=f"mb_b{b + 1}"
                    )
                    nc.sync.dma_start(mb_next[:, :], maskb_d[b + 1 : b + 2, :])

                # main matmul + tanh + scores; next batch's keys transposes
                # interleave with the m-tiles
                psc = [
                    sm_psum.tile([1, NH], f32, tag="sm", name=f"psc{b}_{n}")
                    for n in range(2)
                ]
                for m in range(NC_):
                    ps = ps_pool.tile([P, T], f32, tag="ps", name=f"ps{b}_{m}")
                    for k in range(NC_):
                        for n in range(2):
                            nc.tensor.matmul(
                                ps[:, n * NH : (n + 1) * NH],
                                lhsT=wk[:, k, m * P : (m + 1) * P],
                                rhs=ktr_cur[:, k, n * NH : (n + 1) * NH],
                                start=(k == 0),
                                stop=(k == NC_ - 1),
                            )
                    s = s_pool.tile([P, T], f32r, tag="s", name=f"s{b}_{m}")
                    nc.scalar.activation(
                        s[:, :], ps[:, :], ACT.Tanh, bias=qT[:, m, b : b + 1]
                    )
                    for n in range(2):
                        nc.tensor.matmul(
                            psc[n][:, :],
                            lhsT=wsc[:, m : m + 1],
                            rhs=s[:, n * NH : (n + 1) * NH],
                            start=(m == 0),
                            stop=(m == NC_ - 1),
                        )
                    if not last:
                        kn = emit_keys_chunk(b + 1, m)
                        emit_transposes(f"r{rep}b{b + 1}", kn[:, :], m, ktr_next)

                # mask add (PSUM->SBUF) + softmax on one partition
                sc = row_pool.tile([1, T], f32, tag="row", name=f"sc{b}")
                for n in range(2):
                    nc.vector.tensor_add(
                        sc[:, n * NH : (n + 1) * NH],
                        psc[n][:, :],
                        mb_cur[:, n * NH : (n + 1) * NH],
                    )
                nmax = small_pool.tile([1, 1], f32, tag="nmax", name=f"nmax{b}")
                nc.vector.tensor_reduce(
                    nmax[:, :], sc[:, :], axis=AX.X, op=ALU.max, negate=True
                )
                arow = row_pool.tile([1, T], f32, tag="row", name=f"arow{b}")
                ssum = small_pool.tile([1, 1], f32, tag="ssum", name=f"ssum{b}")
                nc.scalar.activation(
                    arow[:, :], sc[:, :], ACT.Exp, bias=nmax[:, :], accum_out=ssum[:, :]
                )
                rinv = small_pool.tile([1, 1], f32, tag="rinv", name=f"rinv{b}")
                nc.vector.reciprocal(rinv[:, :], ssum[:, :])
                nc.vector.tensor_scalar_mul(arow[:, :], arow[:, :], rinv[:, :])
                nc.sync.dma_start(alph_d[b : b + 1, :], arow[:, :])

                # alphas row -> columns
                paT = tr_psum.tile([P, TB], f32, tag="tr", name=f"paT{b}")
                for k in range(NC_):
                    nc.tensor.transpose(
                        paT[:, k : k + 1],
                        arow[0:1, k * P : (k + 1) * P],
                        ident[0:1, 0:1],
                    )
                aT = small_pool.tile([P, NC_], f32r, tag="aT", name=f"aT{b}")
                nc.vector.tensor_copy(aT[:, :], paT[:, :])

                # context
                pcx = [
                    sm_psum.tile([1, NH], f32, tag="sm", name=f"pcx{b}_{n}")
                    for n in range(2)
                ]
                for k in range(NC_):
                    for n in range(2):
                        nc.tensor.matmul(
                            pcx[n][:, :],
                            lhsT=aT[:, k : k + 1],
                            rhs=vts[k][:, n * NH : (n + 1) * NH],
                            start=(k == 0),
                            stop=(k == NC_ - 1),
                        )
                cxr = row_pool.tile([1, T], f32, tag="row", name=f"cxr{b}")
                for n in range(2):
                    nc.scalar.copy(cxr[:, n * NH : (n + 1) * NH], pcx[n][:, :])
                nc.sync.dma_start(ctx_d[b : b + 1, :], cxr[0:1, :H])

                if not last:
                    ktr_cur = ktr_next
                    mb_cur = mb_next

    if split_drains:
        _split_drain_waits(nc)
    return nc


_NC_CACHE = None


def _get_nc():
    global _NC_CACHE
    if _NC_CACHE is None:
        _NC_CACHE = build_bahdanau_nc()
    return _NC_CACHE


def make_in_maps(query, mask, values, keys, W_key, W_query, w_score):
    """Shard full inputs into per-core input maps (host-side layout only)."""
    query = np.ascontiguousarray(np.asarray(query, dtype=np.float32))
    mask = np.asarray(mask)
    values = np.ascontiguousarray(np.asarray(values, dtype=np.float32))
    keys = np.ascontiguousarray(np.asarray(keys, dtype=np.float32))
    W_key = np.ascontiguousarray(np.asarray(W_key, dtype=np.float32))
    W_query = np.ascontiguousarray(np.asarray(W_query, dtype=np.float32))
    w_score = np.ascontiguousarray(np.asarray(w_score, dtype=np.float32))

    B = query.shape[0]
    n_cores = B // TB
    maskb = np.where(mask, np.float32(0.0), np.float32(-1e30)).astype(np.float32)
    wsc_in = np.ascontiguousarray(w_score.reshape(NC_, P).T)

    in_maps = []
    for c in range(n_cores):
        sl = slice(c * TB, (c + 1) * TB)
        qt = query[sl, 0, :].T  # [QS, TB]
        qtin = np.ascontiguousarray(qt.reshape(NC_, P, TB).transpose(1, 0, 2))
        in_maps.append(
            {
                "keys": keys[sl],
                "values": values[sl],
                "wkey": W_key,
                "wquery": W_query,
                "qtin": qtin,
                "wsc": wsc_in,
                "maskb": np.ascontiguousarray(maskb[sl]),
            }
        )
    return in_maps


def kernel(query, mask, values, keys, W_key, W_query, w_score):
    from concourse.bass_utils import run_bass_kernel_spmd

    B = np.asarray(query).shape[0]
    n_cores = B // TB
    in_maps = make_in_maps(query, mask, values, keys, W_key, W_query, w_score)
    nc = _get_nc()
    res = run_bass_kernel_spmd(nc, in_maps, core_ids=list(range(n_cores)))
    context = np.concatenate([r["ctx"] for r in res.results], axis=0)
    alphas = np.concatenate([r["alph"] for r in res.results], axis=0)
    return context.reshape(B, 1, H), alphas.reshape(B, 1, T)
